# revision 12
# baseline (speedup 1.0000x reference)
"""Trainium2 Bass kernel for nn_NodeBlock (GNN message passing).

Algorithm (8 NeuronCores, SPMD):
  - Edges sharded across cores (200704 padded per core = 1568 tiles of 128).
  - Segment-sum per core into 8 privatized DRAM tables [100352, 52] f32
    (features 0-47, count col 48). Tile t goes to table t%8 via the serial
    gather -> selection-matrix combine (PE) -> scatter chain using indirect
    DMA (dup-safe: within-tile duplicates pre-combined by matmul; across
    tiles the per-table chain is serialized by data deps; tables are
    disjoint across chains).
  - Merge 8 tables (DVE adds) + PE transpose -> f32 bounce [8, 49, 12500]
    -> ReduceScatter(add) -> each core owns summed features for its 12500
    nodes.
  - Node-sharded MLP: agg = sums/max(cnt,1); h = relu(x@W1+b1'); out =
    h@W2+b2, with g@W1g folded into b1'. Output [128, 12500] shard.

Host side only reshapes/slices/concatenates and converts dtypes.
"""
import sys
sys.path.insert(0, '/opt/trn_rl_repo')
import numpy as np

from concourse import bass, bacc, tile, mybir
from concourse.masks import make_identity

dt = mybir.dt
Alu = mybir.AluOpType

P = 8                  # cores
N = 100000             # nodes
E = 1600000            # edges
DE, DN, DG, H, DO = 48, 128, 64, 256, 128
RW = 52                # edge row width: 48 feat + count + recv + pad2
EC = E // P            # 200000 edges per core
NTILE = 1568           # tiles of 128 edges (pad to 200704)
EPAD = NTILE * 128
NCHAIN = 8             # privatized tables
NROWS = 100352         # table rows (= 98 * 53248 / 52, zero-fill friendly)
NS = N // P            # 12500 nodes per core (MLP shard)
NT = 500               # MLP tile (25 tiles of 500 nodes)

_RUNNER = None


def build():
    nc = bacc.Bacc(None, target_bir_lowering=False, debug=False)

    # ---------------- parameters ----------------
    edge_rows = nc.declare_dram_parameter("edge_rows", [EPAD, RW], dt.float32, isOutput=False)
    nodeT = nc.declare_dram_parameter("nodeT", [DN, NS], dt.float32, isOutput=False)
    W1 = nc.declare_dram_parameter("W1", [DE + DN + DG, H], dt.float32, isOutput=False)
    b1 = nc.declare_dram_parameter("b1", [H], dt.float32, isOutput=False)
    W2 = nc.declare_dram_parameter("W2", [H, DO], dt.float32, isOutput=False)
    b2 = nc.declare_dram_parameter("b2", [DO], dt.float32, isOutput=False)
    gvec = nc.declare_dram_parameter("gvec", [DG], dt.float32, isOutput=False)
    out = nc.declare_dram_parameter("out", [DO, NS], dt.float32, isOutput=True)

    # ---------------- internal DRAM ----------------
    tabs = [nc.dram_tensor(f"tab{k}", [NROWS, RW], dt.float32) for k in range(NCHAIN)]
    zdram = nc.dram_tensor("zdram", [128 * 416], dt.float32)   # 53248 zeros
    bounce_in = nc.dram_tensor("bounce_in", [P, DE + 1, NS], dt.float32)
    bounce_out = nc.dram_tensor("bounce_out", [DE + 1, NS], dt.float32)

    with tile.TileContext(nc) as tc:
        with tc.tile_pool(name="persist", bufs=1) as pp:
            ident = pp.tile([128, 128], dt.float32)
            make_identity(nc, ident[:])

            # ---- zero the tables ----
            ztile = pp.tile([128, 416], dt.float32)
            nc.vector.memset(ztile[:], 0)
            nc.sync.dma_start(
                out=bass.AP(tensor=zdram, ap=[[416, 128], [1, 416]], offset=0),
                in_=ztile[:])
            for k in range(NCHAIN):
                nc.sync.dma_start(
                    out=bass.AP(tensor=tabs[k], ap=[[53248, 98], [1, 53248]], offset=0),
                    in_=bass.AP(tensor=zdram, ap=[[0, 98], [1, 53248]], offset=0))

            # ---- weights to SBUF (persist) ----
            tW1a = pp.tile([DE, H], dt.bfloat16)
            tW1b = pp.tile([DN, H], dt.bfloat16)
            tW2 = pp.tile([H // 2, 2 * DO], dt.bfloat16)
            tb1 = pp.tile([128, 2], dt.float32)
            tb2 = pp.tile([DO, 1], dt.float32)
            with tc.tile_pool(name="wtmp", bufs=1) as wp, \
                 tc.tile_pool(name="wps", bufs=1, space="PSUM") as wps:
                tw1fa = wp.tile([DE, H], dt.float32, tag="wfa")
                nc.sync.dma_start(out=tw1fa[:], in_=W1[:DE, :])
                nc.vector.tensor_copy(out=tW1a[:], in_=tw1fa[:])
                tw1fb = wp.tile([DN, H], dt.float32, tag="wfb")
                nc.sync.dma_start(out=tw1fb[:], in_=W1[DE:DE + DN, :])
                nc.vector.tensor_copy(out=tW1b[:], in_=tw1fb[:])
                tw1fg = wp.tile([DG, H], dt.float32, tag="wfg")
                nc.sync.dma_start(out=tw1fg[:], in_=W1[DE + DN:, :])
                tw1g = wp.tile([DG, H], dt.bfloat16)
                nc.vector.tensor_copy(out=tw1g[:], in_=tw1fg[:])
                tw2f = wp.tile([H // 2, DO], dt.float32, tag="w2f")
                nc.sync.dma_start(out=tw2f[:], in_=W2[:H // 2, :])
                nc.vector.tensor_copy(out=tW2[:, :DO], in_=tw2f[:])
                tw2f2 = wp.tile([H // 2, DO], dt.float32, tag="w2f2")
                nc.sync.dma_start(out=tw2f2[:], in_=W2[H // 2:, :])
                nc.vector.tensor_copy(out=tW2[:, DO:], in_=tw2f2[:])
                tb1r = wp.tile([128, 2], dt.float32)
                nc.sync.dma_start(
                    out=tb1r[:],
                    in_=bass.AP(tensor=b1, ap=[[1, 128], [128, 2]], offset=0))
                tgv = wp.tile([DG, 1], dt.bfloat16)
                tgvf = wp.tile([DG, 1], dt.float32)
                nc.sync.dma_start(
                    out=tgvf[:], in_=bass.AP(tensor=gvec, ap=[[1, DG], [0, 1]], offset=0))
                nc.vector.tensor_copy(out=tgv[:], in_=tgvf[:])
                nc.sync.dma_start(
                    out=tb2[:], in_=bass.AP(tensor=b2, ap=[[1, DO], [0, 1]], offset=0))
                for hh in range(2):
                    pg = wps.tile([128, 1], dt.float32, tag="pg")
                    nc.tensor.matmul(out=pg[:], lhsT=tw1g[:, hh * 128:(hh + 1) * 128],
                                     rhs=tgv[:], start=True, stop=True)
                    nc.vector.tensor_tensor(out=tb1[:, hh:hh + 1], in0=tb1r[:, hh:hh + 1],
                                            in1=pg[:], op=Alu.add)

            # ---- stage 1: 8-chain gather/combine/scatter ----
            with tc.tile_pool(name="s1", bufs=24) as s1, \
                 tc.tile_pool(name="ps1", bufs=3, space="PSUM") as ps1:
                for t in range(NTILE):
                    tab = tabs[t % NCHAIN]
                    rows = s1.tile([128, RW], dt.float32, tag="rows")
                    nc.sync.dma_start(out=rows[:], in_=edge_rows[t * 128:(t + 1) * 128, :])
                    idx32 = s1.tile([128, 1], dt.int32, tag="idx32")
                    nc.vector.tensor_copy(out=idx32[:], in_=rows[:, 49:50])
                    tps = ps1.tile([128, 128], dt.float32, tag="tps")
                    nc.tensor.transpose(out=tps[:], in_=rows[:, 49:50].to_broadcast([128, 128]),
                                        identity=ident[:])
                    idxT = s1.tile([128, 128], dt.float32, tag="idxT")
                    nc.scalar.activation(out=idxT[:], in_=tps[:],
                                         func=mybir.ActivationFunctionType.Copy)
                    S = s1.tile([128, 128], dt.float32, tag="S")
                    nc.vector.tensor_tensor(out=S[:], in0=rows[:, 49:50].to_broadcast([128, 128]),
                                            in1=idxT[:], op=Alu.is_equal)
                    acc = s1.tile([128, RW], dt.float32, tag="acc")
                    nc.gpsimd.indirect_dma_start(
                        out=acc[:], out_offset=None, in_=tab[:],
                        in_offset=bass.IndirectOffsetOnAxis(ap=idx32[:, :1], axis=0))
                    cps = ps1.tile([128, RW], dt.float32, tag="cps")
                    nc.tensor.matmul(out=cps[:], lhsT=S[:], rhs=rows[:], start=True, stop=True)
                    nc.vector.tensor_tensor(out=acc[:], in0=acc[:], in1=cps[:], op=Alu.add)
                    nc.gpsimd.indirect_dma_start(
                        out=tab[:],
                        out_offset=bass.IndirectOffsetOnAxis(ap=idx32[:, :1], axis=0),
                        in_=acc[:], in_offset=None)

            # ---- merge 8 tables + transpose -> bounce_in [8, 49, 12500] ----
            with tc.tile_pool(name="mg", bufs=4) as mg, \
                 tc.tile_pool(name="ps2", bufs=4, space="PSUM") as ps2:
                BM = 4           # 4 row-tiles (512 rows) per merge step
                for i in range(N // (128 * BM) + 1):        # 196 steps covers 100352
                    r0 = i * 128 * BM
                    if r0 >= N:
                        break
                    macc = mg.tile([128, BM * RW], dt.float32, tag="macc")
                    nc.sync.dma_start(
                        out=macc[:],
                        in_=bass.AP(tensor=tabs[0],
                                    ap=[[RW, 128], [128 * RW, BM], [1, RW]],
                                    offset=r0 * RW))
                    for k in range(1, NCHAIN):
                        mt = mg.tile([128, BM * RW], dt.float32, tag="mt")
                        nc.sync.dma_start(
                            out=mt[:],
                            in_=bass.AP(tensor=tabs[k],
                                        ap=[[RW, 128], [128 * RW, BM], [1, RW]],
                                        offset=r0 * RW))
                        nc.vector.tensor_tensor(out=macc[:], in0=macc[:], in1=mt[:], op=Alu.add)
                    for q in range(BM):
                        rr = r0 + q * 128
                        if rr >= N:
                            break
                        tp2 = ps2.tile([RW, 128], dt.float32, tag="tp2")
                        nc.tensor.transpose(out=tp2[:], in_=macc[:, q * RW:(q + 1) * RW],
                                            identity=ident[:])
                        stg = mg.tile([DE + 1, 128], dt.float32, tag="stg")
                        nc.scalar.activation(out=stg[:], in_=tp2[:DE + 1, :],
                                             func=mybir.ActivationFunctionType.Copy)
                        # write to bounce_in, splitting at owner boundaries
                        done = 0
                        while done < 128 and rr + done < N:
                            c2 = (rr + done) // NS
                            n = min(128 - done, (c2 + 1) * NS - (rr + done), N - (rr + done))
                            nc.sync.dma_start(
                                out=bounce_in[c2, :, rr + done - c2 * NS:
                                              rr + done - c2 * NS + n],
                                in_=stg[:, done:done + n])
                            done += n

            # ---- ReduceScatter ----
            nc.gpsimd.collective_compute(
                "ReduceScatter", Alu.add,
                replica_groups=[list(range(P))],
                ins=[bounce_in[:]],
                outs=[bounce_out[:]],
            )

            # ---- MLP over owned nodes ----
            sums = pp.tile([DE + 1, NS], dt.float32)
            nc.sync.dma_start(out=sums[:], in_=bounce_out[:])
            trec = pp.tile([1, NS], dt.float32)
            nc.sync.dma_start(out=trec[:], in_=sums[DE:DE + 1, :])
            recip = trec[:, :]
            nc.vector.tensor_scalar(out=recip, in0=recip, scalar1=1.0,
                                    scalar2=None, op0=Alu.max)
            nc.vector.reciprocal(out=recip, in_=recip)

            ones48 = pp.tile([1, DE], dt.float32)
            nc.vector.memset(ones48[:], 1.0)
            with tc.tile_pool(name="m2", bufs=3) as m2, \
                 tc.tile_pool(name="ps3", bufs=2, space="PSUM") as ps3:
                for tt in range(NS // NT):
                    slt = slice(tt * NT, (tt + 1) * NT)
                    # replicate recip across DE partitions via K=1 outer product
                    rep = ps3.tile([DE, NT], dt.float32, tag="rep")
                    nc.tensor.matmul(out=rep[:], lhsT=ones48[:], rhs=recip[:, slt],
                                     start=True, stop=True)
                    agg = m2.tile([DE, NT], dt.bfloat16, tag="agg")
                    nc.vector.tensor_tensor(out=agg[:], in0=sums[:DE, slt],
                                            in1=rep[:], op=Alu.mult)
                    ndf = m2.tile([DN, NT], dt.float32, tag="ndf")
                    nc.sync.dma_start(out=ndf[:], in_=nodeT[:, slt])
                    ndb = m2.tile([DN, NT], dt.bfloat16, tag="ndb")
                    nc.vector.tensor_copy(out=ndb[:], in_=ndf[:])
                    hts = []
                    for hh in range(2):
                        ph = ps3.tile([128, NT], dt.float32, tag="ph")
                        nc.tensor.matmul(out=ph[:], lhsT=tW1a[:, hh * 128:(hh + 1) * 128],
                                         rhs=agg[:], start=True, stop=False)
                        nc.tensor.matmul(out=ph[:], lhsT=tW1b[:, hh * 128:(hh + 1) * 128],
                                         rhs=ndb[:], start=False, stop=True)
                        ht = m2.tile([128, NT], dt.bfloat16, tag=f"ht{hh}")
                        nc.scalar.activation(out=ht[:], in_=ph[:],
                                             func=mybir.ActivationFunctionType.Relu,
                                             bias=tb1[:, hh:hh + 1])
                        hts.append(ht)
                    po = ps3.tile([DO, NT], dt.float32, tag="po")
                    nc.tensor.matmul(out=po[:], lhsT=tW2[:, :DO], rhs=hts[0][:],
                                     start=True, stop=False)
                    nc.tensor.matmul(out=po[:], lhsT=tW2[:, DO:], rhs=hts[1][:],
                                     start=False, stop=True)
                    ot = m2.tile([DO, NT], dt.float32, tag="ot")
                    nc.vector.tensor_scalar(out=ot[:], in0=po[:], scalar1=tb2[:],
                                            scalar2=None, op0=Alu.add)
                    nc.sync.dma_start(out=out[:, slt], in_=ot[:])

    nc.finalize()
    return nc


def _prep_inputs(edge_data, receivers, node_data, global_data, W1, b1, W2, b2):
    """Pure layout/dtype transforms. Returns per-core in_maps."""
    recv = np.ascontiguousarray(receivers).astype(np.float32)
    ed = np.asarray(edge_data, np.float32)
    in_maps = []
    for c in range(P):
        sl = slice(c * EC, (c + 1) * EC)
        er = np.zeros((EPAD, RW), np.float32)
        er[:EC, :DE] = ed[sl]
        er[:EC, DE] = 1.0                       # count col; pad rows stay 0
        er[:EC, DE + 1] = recv[sl]              # receiver as f32; pad -> row 0
        nT = np.ascontiguousarray(node_data[c * NS:(c + 1) * NS].T)
        in_maps.append(dict(
            edge_rows=er, nodeT=nT,
            W1=np.ascontiguousarray(W1, np.float32),
            b1=np.ascontiguousarray(b1, np.float32),
            W2=np.ascontiguousarray(W2, np.float32),
            b2=np.ascontiguousarray(b2, np.float32),
            gvec=np.ascontiguousarray(global_data, np.float32),
        ))
    return in_maps


def kernel(edge_data, receivers, node_data, global_data, W1, b1, W2, b2):
    global _RUNNER
    if _RUNNER is None:
        from bassrun import make_runner
        nc = build()
        _RUNNER = make_runner(nc, P)[0]
    in_maps = _prep_inputs(edge_data, receivers, node_data, global_data, W1, b1, W2, b2)
    results, _ = _RUNNER(in_maps, n_timed=0)
    outT = np.concatenate([results[c]["out"] for c in range(P)], axis=1)  # [DO, N]
    return np.ascontiguousarray(outT.T)


# revision 13
# speedup vs baseline: 11.4509x; 11.4509x over previous
"""Trainium2 Bass kernel for nn_NodeBlock (GNN message passing).

Algorithm (8 NeuronCores, SPMD):
  - Edges sharded across cores (200704 padded per core = 1568 tiles of 128).
  - Segment-sum per core into 8 privatized DRAM tables [100352, 52] f32
    (features 0-47, count col 48). Tile t goes to table t%8 via the serial
    gather -> selection-matrix combine (PE) -> scatter chain using indirect
    DMA (dup-safe: within-tile duplicates pre-combined by matmul; across
    tiles the per-table chain is serialized by data deps; tables are
    disjoint across chains).
  - Merge 8 tables (DVE adds) + PE transpose -> f32 bounce [8, 49, 12500]
    -> ReduceScatter(add) -> each core owns summed features for its 12500
    nodes.
  - Node-sharded MLP: agg = sums/max(cnt,1); h = relu(x@W1+b1'); out =
    h@W2+b2, with g@W1g folded into b1'. Output [128, 12500] shard.

Host side only reshapes/slices/concatenates and converts dtypes.
"""
import sys
sys.path.insert(0, '/opt/trn_rl_repo')
import numpy as np

from concourse import bass, bacc, tile, mybir
from concourse.masks import make_identity

dt = mybir.dt
Alu = mybir.AluOpType

P = 8                  # cores
N = 100000             # nodes
E = 1600000            # edges
DE, DN, DG, H, DO = 48, 128, 64, 256, 128
RW = 52                # edge row width: 48 feat + count + recv + pad2
EC = E // P            # 200000 edges per core
NTILE = 1568           # tiles of 128 edges (pad to 200704)
EPAD = NTILE * 128
NCHAIN = 8             # privatized tables
NROWS = 100352         # table rows (= 98 * 53248 / 52, zero-fill friendly)
NS = N // P            # 12500 nodes per core (MLP shard)
NT = 500               # MLP tile (25 tiles of 500 nodes)

_RUNNER = None


def build():
    nc = bacc.Bacc(None, target_bir_lowering=False, debug=False)

    # ---------------- parameters ----------------
    edge_rows = nc.declare_dram_parameter("edge_rows", [EPAD, RW], dt.float32, isOutput=False)
    nodeT = nc.declare_dram_parameter("nodeT", [DN, NS], dt.float32, isOutput=False)
    W1 = nc.declare_dram_parameter("W1", [DE + DN + DG, H], dt.float32, isOutput=False)
    b1 = nc.declare_dram_parameter("b1", [H], dt.float32, isOutput=False)
    W2 = nc.declare_dram_parameter("W2", [H, DO], dt.float32, isOutput=False)
    b2 = nc.declare_dram_parameter("b2", [DO], dt.float32, isOutput=False)
    gvec = nc.declare_dram_parameter("gvec", [DG], dt.float32, isOutput=False)
    out = nc.declare_dram_parameter("out", [DO, NS], dt.float32, isOutput=True)

    # ---------------- internal DRAM ----------------
    tabs = [nc.dram_tensor(f"tab{k}", [NROWS, RW], dt.float32) for k in range(NCHAIN)]
    zdram = nc.dram_tensor("zdram", [128 * 416], dt.float32)   # 53248 zeros
    bounce_in = nc.dram_tensor("bounce_in", [P, DE + 1, NS], dt.float32)
    bounce_out = nc.dram_tensor("bounce_out", [DE + 1, NS], dt.float32)

    with tile.TileContext(nc) as tc:
        with tc.tile_pool(name="persist", bufs=1) as pp:
            ident = pp.tile([128, 128], dt.float32)
            make_identity(nc, ident[:])

            # ---- zero the tables ----
            ztile = pp.tile([128, 416], dt.float32)
            nc.vector.memset(ztile[:], 0)
            nc.sync.dma_start(
                out=bass.AP(tensor=zdram, ap=[[416, 128], [1, 416]], offset=0),
                in_=ztile[:])
            for k in range(NCHAIN):
                nc.sync.dma_start(
                    out=bass.AP(tensor=tabs[k], ap=[[53248, 98], [1, 53248]], offset=0),
                    in_=bass.AP(tensor=zdram, ap=[[0, 98], [1, 53248]], offset=0))

            # ---- weights to SBUF (persist) ----
            tW1a = pp.tile([DE, H], dt.bfloat16)
            tW1b = pp.tile([DN, H], dt.bfloat16)
            tW2 = pp.tile([H // 2, 2 * DO], dt.bfloat16)
            tb1 = pp.tile([128, 2], dt.float32)
            tb2 = pp.tile([DO, 1], dt.float32)
            with tc.tile_pool(name="wtmp", bufs=1) as wp, \
                 tc.tile_pool(name="wps", bufs=1, space="PSUM") as wps:
                tw1fa = wp.tile([DE, H], dt.float32, tag="wfa")
                nc.sync.dma_start(out=tw1fa[:], in_=W1[:DE, :])
                nc.vector.tensor_copy(out=tW1a[:], in_=tw1fa[:])
                tw1fb = wp.tile([DN, H], dt.float32, tag="wfb")
                nc.sync.dma_start(out=tw1fb[:], in_=W1[DE:DE + DN, :])
                nc.vector.tensor_copy(out=tW1b[:], in_=tw1fb[:])
                tw1fg = wp.tile([DG, H], dt.float32, tag="wfg")
                nc.sync.dma_start(out=tw1fg[:], in_=W1[DE + DN:, :])
                tw1g = wp.tile([DG, H], dt.bfloat16)
                nc.vector.tensor_copy(out=tw1g[:], in_=tw1fg[:])
                tw2f = wp.tile([H // 2, DO], dt.float32, tag="w2f")
                nc.sync.dma_start(out=tw2f[:], in_=W2[:H // 2, :])
                nc.vector.tensor_copy(out=tW2[:, :DO], in_=tw2f[:])
                tw2f2 = wp.tile([H // 2, DO], dt.float32, tag="w2f2")
                nc.sync.dma_start(out=tw2f2[:], in_=W2[H // 2:, :])
                nc.vector.tensor_copy(out=tW2[:, DO:], in_=tw2f2[:])
                tb1r = wp.tile([128, 2], dt.float32)
                nc.sync.dma_start(
                    out=tb1r[:],
                    in_=bass.AP(tensor=b1, ap=[[1, 128], [128, 2]], offset=0))
                tgv = wp.tile([DG, 1], dt.bfloat16)
                tgvf = wp.tile([DG, 1], dt.float32)
                nc.sync.dma_start(
                    out=tgvf[:], in_=bass.AP(tensor=gvec, ap=[[1, DG], [0, 1]], offset=0))
                nc.vector.tensor_copy(out=tgv[:], in_=tgvf[:])
                nc.sync.dma_start(
                    out=tb2[:], in_=bass.AP(tensor=b2, ap=[[1, DO], [0, 1]], offset=0))
                for hh in range(2):
                    pg = wps.tile([128, 1], dt.float32, tag="pg")
                    nc.tensor.matmul(out=pg[:], lhsT=tw1g[:, hh * 128:(hh + 1) * 128],
                                     rhs=tgv[:], start=True, stop=True)
                    nc.vector.tensor_tensor(out=tb1[:, hh:hh + 1], in0=tb1r[:, hh:hh + 1],
                                            in1=pg[:], op=Alu.add)

            # ---- stage 1: 8-chain gather/combine/scatter ----
            with tc.tile_pool(name="s1", bufs=24) as s1, \
                 tc.tile_pool(name="ps1", bufs=3, space="PSUM") as ps1:
                for t in range(NTILE):
                    tab = tabs[t % NCHAIN]
                    rows = s1.tile([128, RW], dt.float32, tag="rows")
                    nc.sync.dma_start(out=rows[:], in_=edge_rows[t * 128:(t + 1) * 128, :])
                    idx32 = s1.tile([128, 1], dt.int32, tag="idx32")
                    nc.vector.tensor_copy(out=idx32[:], in_=rows[:, 49:50])
                    tps = ps1.tile([128, 128], dt.float32, tag="tps")
                    nc.tensor.transpose(out=tps[:], in_=rows[:, 49:50].to_broadcast([128, 128]),
                                        identity=ident[:])
                    idxT = s1.tile([128, 128], dt.float32, tag="idxT")
                    nc.scalar.activation(out=idxT[:], in_=tps[:],
                                         func=mybir.ActivationFunctionType.Copy)
                    S = s1.tile([128, 128], dt.float32, tag="S")
                    nc.vector.tensor_tensor(out=S[:], in0=rows[:, 49:50].to_broadcast([128, 128]),
                                            in1=idxT[:], op=Alu.is_equal)
                    acc = s1.tile([128, RW], dt.float32, tag="acc")
                    nc.gpsimd.indirect_dma_start(
                        out=acc[:], out_offset=None, in_=tab[:],
                        in_offset=bass.IndirectOffsetOnAxis(ap=idx32[:, :1], axis=0))
                    cps = ps1.tile([128, RW], dt.float32, tag="cps")
                    nc.tensor.matmul(out=cps[:], lhsT=S[:], rhs=rows[:], start=True, stop=True)
                    nc.vector.tensor_tensor(out=acc[:], in0=acc[:], in1=cps[:], op=Alu.add)
                    nc.gpsimd.indirect_dma_start(
                        out=tab[:],
                        out_offset=bass.IndirectOffsetOnAxis(ap=idx32[:, :1], axis=0),
                        in_=acc[:], in_offset=None)

            # ---- merge 8 tables + transpose -> bounce_in [8, 49, 12500] ----
            with tc.tile_pool(name="mg", bufs=4) as mg, \
                 tc.tile_pool(name="ps2", bufs=4, space="PSUM") as ps2:
                BM = 4           # 4 row-tiles (512 rows) per merge step
                for i in range(N // (128 * BM) + 1):        # 196 steps covers 100352
                    r0 = i * 128 * BM
                    if r0 >= N:
                        break
                    macc = mg.tile([128, BM * RW], dt.float32, tag="macc")
                    nc.sync.dma_start(
                        out=macc[:],
                        in_=bass.AP(tensor=tabs[0],
                                    ap=[[RW, 128], [128 * RW, BM], [1, RW]],
                                    offset=r0 * RW))
                    for k in range(1, NCHAIN):
                        mt = mg.tile([128, BM * RW], dt.float32, tag="mt")
                        nc.sync.dma_start(
                            out=mt[:],
                            in_=bass.AP(tensor=tabs[k],
                                        ap=[[RW, 128], [128 * RW, BM], [1, RW]],
                                        offset=r0 * RW))
                        nc.vector.tensor_tensor(out=macc[:], in0=macc[:], in1=mt[:], op=Alu.add)
                    for q in range(BM):
                        rr = r0 + q * 128
                        if rr >= N:
                            break
                        tp2 = ps2.tile([RW, 128], dt.float32, tag="tp2")
                        nc.tensor.transpose(out=tp2[:], in_=macc[:, q * RW:(q + 1) * RW],
                                            identity=ident[:])
                        stg = mg.tile([DE + 1, 128], dt.float32, tag="stg")
                        nc.scalar.activation(out=stg[:], in_=tp2[:DE + 1, :],
                                             func=mybir.ActivationFunctionType.Copy)
                        # write to bounce_in, splitting at owner boundaries
                        done = 0
                        while done < 128 and rr + done < N:
                            c2 = (rr + done) // NS
                            n = min(128 - done, (c2 + 1) * NS - (rr + done), N - (rr + done))
                            nc.sync.dma_start(
                                out=bounce_in[c2, :, rr + done - c2 * NS:
                                              rr + done - c2 * NS + n],
                                in_=stg[:, done:done + n])
                            done += n

            # ---- ReduceScatter ----
            nc.gpsimd.collective_compute(
                "ReduceScatter", Alu.add,
                replica_groups=[list(range(P))],
                ins=[bounce_in[:]],
                outs=[bounce_out[:]],
            )

            # ---- MLP over owned nodes ----
            sums = pp.tile([DE + 1, NS], dt.float32)
            nc.sync.dma_start(out=sums[:], in_=bounce_out[:])
            trec = pp.tile([1, NS], dt.float32)
            nc.sync.dma_start(out=trec[:], in_=sums[DE:DE + 1, :])
            recip = trec[:, :]
            nc.vector.tensor_scalar(out=recip, in0=recip, scalar1=1.0,
                                    scalar2=None, op0=Alu.max)
            nc.vector.reciprocal(out=recip, in_=recip)

            ones48 = pp.tile([1, DE], dt.float32)
            nc.vector.memset(ones48[:], 1.0)
            with tc.tile_pool(name="m2", bufs=3) as m2, \
                 tc.tile_pool(name="ps3", bufs=2, space="PSUM") as ps3:
                for tt in range(NS // NT):
                    slt = slice(tt * NT, (tt + 1) * NT)
                    # replicate recip across DE partitions via K=1 outer product
                    rep = ps3.tile([DE, NT], dt.float32, tag="rep")
                    nc.tensor.matmul(out=rep[:], lhsT=ones48[:], rhs=recip[:, slt],
                                     start=True, stop=True)
                    agg = m2.tile([DE, NT], dt.bfloat16, tag="agg")
                    nc.vector.tensor_tensor(out=agg[:], in0=sums[:DE, slt],
                                            in1=rep[:], op=Alu.mult)
                    ndf = m2.tile([DN, NT], dt.float32, tag="ndf")
                    nc.sync.dma_start(out=ndf[:], in_=nodeT[:, slt])
                    ndb = m2.tile([DN, NT], dt.bfloat16, tag="ndb")
                    nc.vector.tensor_copy(out=ndb[:], in_=ndf[:])
                    hts = []
                    for hh in range(2):
                        ph = ps3.tile([128, NT], dt.float32, tag="ph")
                        nc.tensor.matmul(out=ph[:], lhsT=tW1a[:, hh * 128:(hh + 1) * 128],
                                         rhs=agg[:], start=True, stop=False)
                        nc.tensor.matmul(out=ph[:], lhsT=tW1b[:, hh * 128:(hh + 1) * 128],
                                         rhs=ndb[:], start=False, stop=True)
                        ht = m2.tile([128, NT], dt.bfloat16, tag=f"ht{hh}")
                        nc.scalar.activation(out=ht[:], in_=ph[:],
                                             func=mybir.ActivationFunctionType.Relu,
                                             bias=tb1[:, hh:hh + 1])
                        hts.append(ht)
                    po = ps3.tile([DO, NT], dt.float32, tag="po")
                    nc.tensor.matmul(out=po[:], lhsT=tW2[:, :DO], rhs=hts[0][:],
                                     start=True, stop=False)
                    nc.tensor.matmul(out=po[:], lhsT=tW2[:, DO:], rhs=hts[1][:],
                                     start=False, stop=True)
                    ot = m2.tile([DO, NT], dt.float32, tag="ot")
                    nc.vector.tensor_scalar(out=ot[:], in0=po[:], scalar1=tb2[:],
                                            scalar2=None, op0=Alu.add)
                    nc.sync.dma_start(out=out[:, slt], in_=ot[:])

    nc.finalize()
    return nc


def _prep_inputs(edge_data, receivers, node_data, global_data, W1, b1, W2, b2):
    """Pure layout/dtype transforms. Returns per-core in_maps."""
    recv = np.ascontiguousarray(receivers).astype(np.float32)
    ed = np.asarray(edge_data, np.float32)
    in_maps = []
    for c in range(P):
        sl = slice(c * EC, (c + 1) * EC)
        er = np.zeros((EPAD, RW), np.float32)
        er[:EC, :DE] = ed[sl]
        er[:EC, DE] = 1.0                       # count col; pad rows stay 0
        er[:EC, DE + 1] = recv[sl]              # receiver as f32; pad -> row 0
        nT = np.ascontiguousarray(node_data[c * NS:(c + 1) * NS].T)
        in_maps.append(dict(
            edge_rows=er, nodeT=nT,
            W1=np.ascontiguousarray(W1, np.float32),
            b1=np.ascontiguousarray(b1, np.float32),
            W2=np.ascontiguousarray(W2, np.float32),
            b2=np.ascontiguousarray(b2, np.float32),
            gvec=np.ascontiguousarray(global_data, np.float32),
        ))
    return in_maps




# ---------------- inlined PJRT runner (self-contained) ----------------
def make_runner(nc, n_cores):
    import time
    import jax
    from jax.sharding import Mesh, PartitionSpec, NamedSharding
    from jax.experimental.shard_map import shard_map
    from concourse.bass2jax import _bass_exec_p, install_neuronx_cc_hook, partition_id_tensor

    install_neuronx_cc_hook()
    partition_name = nc.partition_id_tensor.name if nc.partition_id_tensor else None
    in_names, out_names, out_avals, zero_outs = [], [], [], []
    for alloc in nc.m.functions[0].allocations:
        if not isinstance(alloc, mybir.MemoryLocationSet):
            continue
        name = alloc.memorylocations[0].name
        if alloc.kind == "ExternalInput":
            if name != partition_name:
                in_names.append(name)
        elif alloc.kind == "ExternalOutput":
            out_names.append(name)
            shape = tuple(alloc.tensor_shape)
            dtype = mybir.dt.np(alloc.dtype)
            out_avals.append(jax.core.ShapedArray(shape, dtype))
            zero_outs.append(np.zeros(shape, dtype))
    n_params = len(in_names)
    all_in_names = in_names + out_names + ([partition_name] if partition_name else [])

    def _body(*args):
        operands = list(args)
        if partition_name is not None:
            operands.append(partition_id_tensor())
        outs = _bass_exec_p.bind(
            *operands,
            out_avals=tuple(out_avals),
            in_names=tuple(all_in_names),
            out_names=tuple(out_names),
            lowering_input_output_aliases=(),
            sim_require_finite=True,
            sim_require_nnan=True,
            nc=nc,
        )
        return tuple(outs)

    devices = jax.devices()[:n_cores]
    mesh = Mesh(np.asarray(devices), ("core",))
    spec = PartitionSpec("core")
    in_specs = (spec,) * (n_params + len(out_names))
    out_specs = (spec,) * len(out_names)
    fn = jax.jit(shard_map(_body, mesh=mesh, in_specs=in_specs,
                           out_specs=out_specs, check_rep=False),
                 keep_unused=True)
    sharding = NamedSharding(mesh, spec)

    def run(in_maps, n_timed=0):
        assert len(in_maps) == n_cores
        concat_in = [
            np.concatenate([np.asarray(in_maps[c][name]) for c in range(n_cores)], axis=0)
            for name in in_names
        ]
        concat_zeros = [
            np.zeros((n_cores * z.shape[0], *z.shape[1:]), z.dtype) for z in zero_outs
        ]
        dev_in = [jax.device_put(a, sharding) for a in concat_in]
        dev_zero = [jax.device_put(a, sharding) for a in concat_zeros]
        t0 = time.time()
        out_arrs = fn(*dev_in, *dev_zero)
        jax.block_until_ready(out_arrs)
        first_s = time.time() - t0
        per_exec = None
        if n_timed > 0:
            t0 = time.time()
            out_arrs = fn(*dev_in, *dev_zero)
            jax.block_until_ready(out_arrs)
            warm_s = time.time() - t0
            ts = []
            for _ in range(n_timed):
                t0 = time.time()
                out_arrs = fn(*dev_in, *dev_zero)
                jax.block_until_ready(out_arrs)
                ts.append(time.time() - t0)
            per_exec = (first_s, warm_s, ts)
        out_np = [np.asarray(a) for a in out_arrs]
        results = [
            {name: out_np[i].reshape(n_cores, *out_avals[i].shape)[c]
             for i, name in enumerate(out_names)}
            for c in range(n_cores)
        ]
        return results, per_exec

    return run, in_names, out_names

def kernel(edge_data, receivers, node_data, global_data, W1, b1, W2, b2):
    global _RUNNER
    if _RUNNER is None:
        nc = build()
        _RUNNER = make_runner(nc, P)[0]
    in_maps = _prep_inputs(edge_data, receivers, node_data, global_data, W1, b1, W2, b2)
    results, _ = _RUNNER(in_maps, n_timed=0)
    outT = np.concatenate([results[c]["out"] for c in range(P)], axis=1)  # [DO, N]
    return np.ascontiguousarray(outT.T)
